# revision 21
# baseline (speedup 1.0000x reference)
"""AdaptiveSpectralFeatureRefinement (Euclidean) — Trainium2 Bass kernel.

Reference op (per batch element b):
  patches = unfold3x3(fused_features)                 # [C, 9, H, W]
  dist_k  = || patches_k - fe_lv ||_2  (over C)       # [9, H, W]
  w       = softmax_k(-dist_k)
  out     = sum_k w_k * patches_k + fe_lv             # [C, H, W]

Sharding: data-parallel over batch B=8 across the 8 NeuronCores.

v3 ("half pipeline"): W split into 2 halves of 64 cols, each with its own
SBUF tiles in [H=128 partitions, (c, w) flat] layout.  The f tiles carry
one guard column per side (host-supplied: real neighbor data at the
interior cut, ZEROS at the global w edges) and the fm/fp (h-shifted)
tiles carry ZERO rows at the global h edges, so out-of-bounds patches
are exactly zero and dist/softmax/weighted-sum need NO edge fixups (the
reference's zero padding falls out naturally).

Per half:
  P1: per dy-group {-1,0,+1}: one fused 3-dx subtract (DVE, overlapping
      stride-1 AP), one fused square (Act), then sum-over-C:
        dy=0 group -> per k: 8 accumulating 512-col identity matmuls
            (PE) -> [8c,64w] PSUM partials -> single-instruction
            tensor_reduce finish over a transposed [w,c] PSUM view (DVE)
        dy=+-1 groups -> fused 3-neighbor bf16 halving tree (DVE): each
            level processes all 3 k's in ONE instruction.
  P2: softmax: min/sum over the 9 neighbors via single tensor_reduce
      ops on a transposed [w,k] view of dist; sqrt/exp on Act.
  P3: products per (dy, c-half) fused over 3 dx (DVE; one on Pool in
      half 0), 10 accumulating matmuls per 16-channel group (PE) incl.
      the fe_lv residual, evacuation on Act, store DMA per half.
The two halves pipeline: engines overlap half-0 P3 with half-1 P1.
"""

import sys

if "/opt/trn_rl_repo" not in sys.path:
    sys.path.insert(0, "/opt/trn_rl_repo")

import os
from contextlib import ExitStack

import numpy as np

import concourse.bass as bass
import concourse.tile as tile
from concourse import mybir
from concourse.ap import AP as _AP
from concourse.masks import make_identity

B, C, H, W = 8, 64, 128, 128
N_CORES = 8
NH = 2                      # w-halves
WH = W // NH                # 64 cols per half
GW = WH + 2                 # stored cols incl 1 guard col each side
FP = mybir.dt.float32
BF = mybir.dt.bfloat16
ACT = mybir.ActivationFunctionType
X = mybir.AxisListType.X


def _split_sync_waits(nc, max_waits=1):
    """This container's walrus codegen accepts at most one sync-wait command
    per instruction; hoist extras into NoOps on the same engine."""
    for f in nc.m.functions:
        for blk in f.blocks:
            new_insts = []
            changed = False
            for inst in blk.instructions:
                si = getattr(inst, "sync_info", None)
                if si is not None and si.on_wait and len(si.on_wait) > max_waits:
                    waits = list(si.on_wait)
                    for i, w in enumerate(waits[max_waits:]):
                        nop = mybir.InstNoOp(
                            name=f"{inst.name}_ws{i}",
                            engine=inst.engine,
                            sync_info=mybir.SyncInfo(on_wait=[w],
                                                     on_update=[]),
                            bass_nofuse=True,
                        )
                        new_insts.append(nop)
                    inst.sync_info = mybir.SyncInfo(
                        on_wait=waits[:max_waits],
                        on_update=list(si.on_update),
                    )
                    changed = True
                new_insts.append(inst)
            if changed:
                blk.instructions = new_insts
    return nc


def _f3(ftile, c0, cg):
    """[128, 3(dx), cg, WH] view of a guarded [128, C, GW] tile: element
    (p, dx, c, w) = ftile[p, c0+c, w+dx+1]; dx and w share stride 1."""
    base = ftile[:, c0:c0 + cg, 1:1 + WH]
    bap = [list(x) for x in list(base.ap)]
    return _AP(base.tensor, base.offset - 1,
               [bap[0], [1, 3], bap[1], bap[2]])


def _tview(ap3, inner):
    """Swap the two free dims of a [128, A, B] AP -> [128, B, A] so that
    tensor_reduce(axis=X) reduces the original MIDDLE axis."""
    dims = [list(x) for x in list(ap3.ap)]
    assert len(dims) == 3
    return _AP(ap3.tensor, ap3.offset, [dims[0], dims[2], dims[1]])


def _build_kernel(split_waits=True):
    nc = bass.Bass("TRN2", target_bir_lowering=False, debug=False,
                   num_devices=N_CORES)

    x_d = [nc.dram_tensor(f"x{h}", [H, C * WH], BF, kind="ExternalInput").ap()
           for h in range(NH)]
    f_d = [nc.dram_tensor(f"f{h}", [H, C * GW], BF, kind="ExternalInput").ap()
           for h in range(NH)]
    fm_d = [nc.dram_tensor(f"fm{h}", [H, C * GW], BF,
                           kind="ExternalInput").ap() for h in range(NH)]
    fp_d = [nc.dram_tensor(f"fp{h}", [H, C * GW], BF,
                           kind="ExternalInput").ap() for h in range(NH)]
    o_d = [nc.dram_tensor(f"o{h}", [H, C * WH], BF,
                          kind="ExternalOutput").ap() for h in range(NH)]

    with tile.TileContext(nc) as tc, ExitStack() as ctx:
        main = ctx.enter_context(tc.tile_pool(name="main", bufs=1))
        tp = ctx.enter_context(tc.tile_pool(name="tp", bufs=3))
        tp2 = ctx.enter_context(tc.tile_pool(name="tp2", bufs=5))
        fin = ctx.enter_context(tc.tile_pool(name="fin", bufs=1))
        pst = ctx.enter_context(tc.tile_pool(name="pst", bufs=2,
                                             space="PSUM"))
        psa = ctx.enter_context(tc.tile_pool(name="psa", bufs=3,
                                             space="PSUM"))

        ident = main.tile([128, 128], BF)
        make_identity(nc, ident[:, :])

        xh = [main.tile([128, C, WH], BF, name=f"xh{h}") for h in range(NH)]
        fh = [main.tile([128, C, GW], BF, name=f"fh{h}") for h in range(NH)]
        fmh = [main.tile([128, C, GW], BF, name=f"fmh{h}") for h in range(NH)]
        fph = [main.tile([128, C, GW], BF, name=f"fph{h}") for h in range(NH)]
        outh = [main.tile([128, C, WH], BF, name=f"outh{h}")
                for h in range(NH)]
        dist = [main.tile([128, 9, WH], FP, name=f"dist{h}")
                for h in range(NH)]
        ewb = [main.tile([128, 9, WH], BF, name=f"ewb{h}") for h in range(NH)]
        mmin = [main.tile([128, WH], FP, name=f"mmin{h}") for h in range(NH)]
        ssum = [main.tile([128, WH], FP, name=f"ssum{h}") for h in range(NH)]

        # ---- loads: half-0 tiles first; 3 trigger queues ----
        # half-0 spread over all 3 trigger queues, c-halved so the first
        # (c-split) subtract can start as soon as the first halves land
        nc.sync.dma_start(out=xh[0][:, 0:32, :], in_=x_d[0][:, 0:32 * WH])
        nc.scalar.dma_start(out=fh[0][:, 0:32, :], in_=f_d[0][:, 0:32 * GW])
        nc.gpsimd.dma_start(out=fmh[0][:, :, :], in_=fm_d[0])
        nc.sync.dma_start(out=xh[0][:, 32:64, :], in_=x_d[0][:, 32 * WH:])
        nc.scalar.dma_start(out=fh[0][:, 32:64, :], in_=f_d[0][:, 32 * GW:])
        nc.gpsimd.dma_start(out=fph[0][:, :, :], in_=fp_d[0])
        nc.sync.dma_start(out=xh[1][:, :, :], in_=x_d[1])
        nc.sync.dma_start(out=fh[1][:, :, :], in_=f_d[1])
        nc.gpsimd.dma_start(out=fmh[1][:, :, :], in_=fm_d[1])
        nc.scalar.dma_start(out=fph[1][:, :, :], in_=fp_d[1])

        f_dy = lambda h: {-1: fmh[h], 0: fh[h], 1: fph[h]}

        def tree_pe(h, k, sqk):
            """sum over C: 8 accumulating 512-col matmuls + 1-op finish."""
            p = pst.tile([128, 8, WH], FP, tag="pst", name="ptree")
            flat = sqk.rearrange("p c w -> p (c w)")
            pf = p[:, :, :].rearrange("p c w -> p (c w)")
            for j in range(8):
                nc.tensor.matmul(
                    pf, ident[:, :],
                    flat[:, j * 8 * WH:(j + 1) * 8 * WH],
                    start=(j == 0), stop=(j == 7),
                )
            nc.vector.tensor_reduce(dist[h][:, k, :], _tview(p[:, :, :], 8),
                                    axis=X, op=mybir.AluOpType.add)

        def tree3_dve(h, k0, sq3):
            """fused 3-neighbor halving tree over c: each level handles all
            3 k's in one instruction; last level writes fp32 dist rows."""
            s = fin.tile([128, 3, 32, WH], BF, tag="s3", name="s3")
            nc.vector.tensor_add(s[:, :, :, :], sq3[:, :, 0:32, :],
                                 sq3[:, :, 32:64, :])
            n = 16
            while n >= 2:
                nc.vector.tensor_add(s[:, :, 0:n, :], s[:, :, 0:n, :],
                                     s[:, :, n:2 * n, :])
                n //= 2
            nc.vector.tensor_add(dist[h][:, k0:k0 + 3, :], s[:, :, 0, :],
                                 s[:, :, 1, :])

        def p1_group(h, dy, split_first=False):
            k0 = (dy + 1) * 3
            t = tp.tile([128, 3, C, WH], BF, tag="t", name="tsub")
            xb = xh[h][:, :, :].unsqueeze(1).broadcast_to([128, 3, C, WH])
            if split_first:
                # c-halved sub+square so compute starts before the full
                # f/x tiles have streamed in
                for cs in range(2):
                    ca, cb = cs * 32, cs * 32 + 32
                    xbh = (xh[h][:, ca:cb, :].unsqueeze(1)
                           .broadcast_to([128, 3, 32, WH]))
                    nc.vector.tensor_sub(t[:, :, ca:cb, :],
                                         _f3(f_dy(h)[dy], ca, 32), xbh)
                    nc.scalar.activation(t[:, :, ca:cb, :],
                                         t[:, :, ca:cb, :], ACT.Square)
            else:
                nc.vector.tensor_sub(t[:, :, :, :], _f3(f_dy(h)[dy], 0, C),
                                     xb)
                # Act squares 56 channels; DVE self-multiplies the last 8
                # (fills DVE's per-group slack, shortens the Act period)
                nc.scalar.activation(t[:, :, 0:56, :], t[:, :, 0:56, :],
                                     ACT.Square)
                nc.vector.tensor_mul(t[:, :, 56:64, :], t[:, :, 56:64, :],
                                     t[:, :, 56:64, :])
            for kk in range(3):
                tree_pe(h, k0 + kk, t[:, kk, :, :])

        def p2(h):
            d, mn, sm = dist[h], mmin[h], ssum[h]
            nc.vector.tensor_reduce(mn[:, :], _tview(d[:, :, :], 9),
                                    axis=X, op=mybir.AluOpType.min)
            nc.scalar.activation(d[:, :, :], d[:, :, :], ACT.Sqrt)
            nc.scalar.activation(mn[:, :], mn[:, :], ACT.Sqrt)
            nc.vector.tensor_sub(
                d[:, :, :],
                mn[:, :].unsqueeze(1).broadcast_to([128, 9, WH]),
                d[:, :, :],
            )
            nc.scalar.activation(d[:, :, :], d[:, :, :], ACT.Exp)
            nc.vector.tensor_reduce(sm[:, :], _tview(d[:, :, :], 9),
                                    axis=X, op=mybir.AluOpType.add)
            nc.vector.reciprocal(sm[:, :], sm[:, :])
            nc.vector.tensor_mul(
                ewb[h][:, :, :], d[:, :, :],
                sm[:, :].unsqueeze(1).broadcast_to([128, 9, WH]),
            )

        def p3(h):
            fd = f_dy(h)
            for g in range(4):               # 16-channel groups
                c0 = g * 16
                t2s = []
                for dyi, dy in enumerate((-1, 0, 1)):
                    k0 = dyi * 3
                    e3 = (ewb[h][:, k0:k0 + 3, :]
                          .unsqueeze(2).broadcast_to([128, 3, 16, WH]))
                    t2 = tp2.tile([128, 3, 16, WH], BF, tag="t2", name="t2")
                    nc.vector.tensor_mul(t2[:, :, :, :],
                                         _f3(fd[dy], c0, 16), e3)
                    t2s.append(t2)
                pacc = psa.tile([128, 16 * WH], FP, tag="pacc", name="pacc")
                # (dyi, kk) outermost so each t2 is fully consumed (and its
                # pool slot released) after its 6 matmuls
                for dyi in range(3):
                    for kk in range(3):
                        for cc in range(2):  # 512-col chunks (8c each)
                            nc.tensor.matmul(
                                pacc[:, cc * 512:(cc + 1) * 512], ident[:, :],
                                (t2s[dyi][:, kk, cc * 8:cc * 8 + 8, :]
                                 .rearrange("p c w -> p (c w)")),
                                start=(dyi == 0 and kk == 0), stop=False,
                            )
                for cc in range(2):
                    nc.tensor.matmul(
                        pacc[:, cc * 512:(cc + 1) * 512], ident[:, :],
                        (xh[h][:, c0 + cc * 8:c0 + cc * 8 + 8, :]
                         .rearrange("p c w -> p (c w)")),
                        start=False, stop=True,
                    )
                nc.scalar.activation(
                    (outh[h][:, c0:c0 + 16, :]
                     .rearrange("p c w -> p (c w)")),
                    pacc[:, :], ACT.Copy)
                nc.sync.dma_start(   # store each 16-channel group ASAP
                    out=o_d[h][:, g * 16 * WH:(g + 1) * 16 * WH],
                    in_=(outh[h][:, g * 16:(g + 1) * 16, :]
                         .rearrange("p c w -> p (c w)")))

        for h in range(NH):
            order = (0, -1, 1) if h == 0 else (-1, 0, 1)
            for i, dy in enumerate(order):
                p1_group(h, dy, split_first=(h == 0 and i == 0))
            p2(h)
            p3(h)

    return _split_sync_waits(nc) if split_waits else nc


_cache = {}


class _SpmdRunner:
    """Executes the Bass graph SPMD on the 8 cores via PJRT/shard_map."""

    def __init__(self, nc, n_cores):
        import jax
        from jax.experimental.shard_map import shard_map
        from jax.sharding import Mesh, NamedSharding, PartitionSpec

        from concourse import bass2jax as b2j

        b2j.install_neuronx_cc_hook()
        self.nc = nc
        self.n_cores = n_cores
        partition_name = (
            nc.partition_id_tensor.name if nc.partition_id_tensor else None
        )

        in_names, out_names, out_avals = [], [], []
        for alloc in nc.m.functions[0].allocations:
            if not isinstance(alloc, mybir.MemoryLocationSet):
                continue
            name = alloc.memorylocations[0].name
            if alloc.kind == "ExternalInput":
                if name != partition_name:
                    in_names.append(name)
            elif alloc.kind == "ExternalOutput":
                out_names.append(name)
                out_avals.append(
                    jax.core.ShapedArray(
                        tuple(alloc.tensor_shape), mybir.dt.np(alloc.dtype)
                    )
                )
        self.in_names, self.out_names = in_names, out_names
        self.out_avals = out_avals
        n_params, n_outs = len(in_names), len(out_names)
        all_in_names = in_names + out_names + (
            [partition_name] if partition_name else []
        )

        def _body(*args):
            operands = list(args)
            if partition_name is not None:
                operands.append(b2j.partition_id_tensor())
            outs = b2j._bass_exec_p.bind(
                *operands,
                out_avals=tuple(out_avals),
                in_names=tuple(all_in_names),
                out_names=tuple(out_names),
                lowering_input_output_aliases=(),
                sim_require_finite=True,
                sim_require_nnan=True,
                nc=nc,
            )
            return tuple(outs)

        self.devices = jax.devices()[:n_cores]
        assert len(self.devices) == n_cores
        mesh = Mesh(np.asarray(self.devices), ("core",))
        self.sharding = NamedSharding(mesh, PartitionSpec("core"))
        self.sharded = jax.jit(
            shard_map(
                _body, mesh=mesh,
                in_specs=(PartitionSpec("core"),) * (n_params + n_outs),
                out_specs=(PartitionSpec("core"),) * n_outs,
                check_rep=False,
            ),
            donate_argnums=tuple(range(n_params, n_params + n_outs)),
            keep_unused=True,
        )

    def _make_global(self, shards_np):
        import jax

        shards = [
            jax.device_put(s, self.devices[c])
            for c, s in enumerate(shards_np)
        ]
        gshape = (self.n_cores * shards_np[0].shape[0],) + tuple(
            shards_np[0].shape[1:]
        )
        return jax.make_array_from_single_device_arrays(
            gshape, self.sharding, shards
        )

    def __call__(self, in_maps):
        gin = [
            self._make_global(
                [np.asarray(in_maps[c][name]) for c in range(self.n_cores)]
            )
            for name in self.in_names
        ]
        gzero = [
            self._make_global(
                [np.zeros(a.shape, a.dtype) for _ in range(self.n_cores)]
            )
            for a in self.out_avals
        ]
        out_arrs = self.sharded(*gin, *gzero)
        results = [dict() for _ in range(self.n_cores)]
        for i, name in enumerate(self.out_names):
            for sh in out_arrs[i].addressable_shards:
                results[self.devices.index(sh.device)][name] = np.asarray(
                    sh.data
                )
        return results




def _get_runner():
    if "runner" not in _cache:
        _cache["runner"] = _SpmdRunner(_build_kernel(), N_CORES)
    return _cache["runner"]


def _prep_inputs(fe_lv, fused_features):
    import ml_dtypes

    bf = ml_dtypes.bfloat16
    fe_lv = np.asarray(fe_lv, dtype=np.float32)
    fused_features = np.asarray(fused_features, dtype=np.float32)
    in_maps = []
    for i in range(N_CORES):
        x = np.ascontiguousarray(fe_lv[i].transpose(1, 0, 2)).astype(bf)
        f = np.ascontiguousarray(
            fused_features[i].transpose(1, 0, 2)).astype(bf)   # [H, C, W]
        # h-shifted variants with ZERO rows at the global h edges
        fm = np.concatenate([np.zeros_like(f[0:1]), f[:-1]], axis=0)
        fp = np.concatenate([f[1:], np.zeros_like(f[-1:])], axis=0)
        # guarded [H, C, W+2] with zero cols at the global w edges
        def _guard(a):
            g = np.zeros((H, C, W + 2), dtype=bf)
            g[:, :, 1:W + 1] = a
            return g
        fg, fmg, fpg = _guard(f), _guard(fm), _guard(fp)
        m = {}
        for h in range(NH):
            w0 = h * WH
            m[f"x{h}"] = np.ascontiguousarray(
                x[:, :, w0:w0 + WH]).reshape(H, C * WH)
            m[f"f{h}"] = np.ascontiguousarray(
                fg[:, :, w0:w0 + GW]).reshape(H, C * GW)
            m[f"fm{h}"] = np.ascontiguousarray(
                fmg[:, :, w0:w0 + GW]).reshape(H, C * GW)
            m[f"fp{h}"] = np.ascontiguousarray(
                fpg[:, :, w0:w0 + GW]).reshape(H, C * GW)
        in_maps.append(m)
    return in_maps


def _post_outputs(results):
    outs = []
    for i in range(N_CORES):
        hs = [results[i][f"o{h}"].reshape(H, C, WH) for h in range(NH)]
        hcw = np.concatenate(hs, axis=2)           # [H, C, W]
        outs.append(hcw.transpose(1, 0, 2))        # [C, H, W]
    return np.ascontiguousarray(np.stack(outs, axis=0)).astype(np.float32)


def kernel(fe_lv, fused_features):
    runner = _get_runner()
    results = runner(_prep_inputs(fe_lv, fused_features))
    return _post_outputs(results)

def bench(fe_lv, fused_features, trace_dir=None):
    """Run once (compiling/warming), then re-run under an NTFF profile
    capture and return (out, exec_time_ns, trace_info)."""
    import ctypes
    import glob as _glob
    import tempfile

    out = kernel(fe_lv, fused_features)
    runner = _cache["runner"]

    neff_dir = trace_dir or tempfile.mkdtemp(prefix="ntff_prof_")
    lib = ctypes.CDLL("/opt/axon/libaxon_pjrt.so")
    if not hasattr(lib, "axon_start_nrt_profile"):
        return out, None, "no axon_start_nrt_profile symbol"
    lib.axon_start_nrt_profile.argtypes = [
        ctypes.POINTER(ctypes.c_int64), ctypes.c_size_t,
    ]
    lib.axon_start_nrt_profile.restype = ctypes.c_int64
    lib.axon_stop_nrt_profile.argtypes = [ctypes.c_char_p]
    lib.axon_stop_nrt_profile.restype = ctypes.c_int64

    in_maps = _prep_inputs(fe_lv, fused_features)
    rc = lib.axon_start_nrt_profile(None, 0)
    if rc != 0:
        return out, None, f"axon_start_nrt_profile rc={rc}"
    runner(in_maps)
    n = lib.axon_stop_nrt_profile(neff_dir.encode())
    if n <= 0:
        return out, None, f"axon_stop_nrt_profile rc={n}"

    ntffs = _glob.glob(os.path.join(neff_dir, "*_body*.ntff"))
    if not ntffs:
        return out, None, f"no *_body*.ntff in {neff_dir}: " + str(
            sorted(os.listdir(neff_dir)))

    import gauge.profiler
    from concourse._compat import FishPath

    profile = gauge.profiler.Profile(
        profile_path=FishPath(neff_dir),
        kernel_dev_mode=True,
        profile_on_exit=False,
        bass_kernel=_cache["runner"].nc.m,
        offline_processing=True,
        fname="*_body*",
    )
    perfetto_results = profile.to_perfetto(model_index=(0,))
    if not perfetto_results:
        return out, None, f"no perfetto results ({neff_dir})"
    pr = perfetto_results[0]
    return out, pr.exec_time_ns, {"trace_path": pr.trace_path,
                                  "neff_dir": neff_dir}


# revision 22
# speedup vs baseline: 1.0043x; 1.0043x over previous
"""AdaptiveSpectralFeatureRefinement (Euclidean) — Trainium2 Bass kernel.

Reference op (per batch element b):
  patches = unfold3x3(fused_features)                 # [C, 9, H, W]
  dist_k  = || patches_k - fe_lv ||_2  (over C)       # [9, H, W]
  w       = softmax_k(-dist_k)
  out     = sum_k w_k * patches_k + fe_lv             # [C, H, W]

Sharding: data-parallel over batch B=8 across the 8 NeuronCores.

v3 ("half pipeline"): W split into 2 halves of 64 cols, each with its own
SBUF tiles in [H=128 partitions, (c, w) flat] layout.  The f tiles carry
one guard column per side (host-supplied: real neighbor data at the
interior cut, ZEROS at the global w edges) and the fm/fp (h-shifted)
tiles carry ZERO rows at the global h edges, so out-of-bounds patches
are exactly zero and dist/softmax/weighted-sum need NO edge fixups (the
reference's zero padding falls out naturally).

Per half:
  P1: per dy-group {-1,0,+1}: one fused 3-dx subtract (DVE, overlapping
      stride-1 AP), one fused square (Act), then sum-over-C:
        dy=0 group -> per k: 8 accumulating 512-col identity matmuls
            (PE) -> [8c,64w] PSUM partials -> single-instruction
            tensor_reduce finish over a transposed [w,c] PSUM view (DVE)
        dy=+-1 groups -> fused 3-neighbor bf16 halving tree (DVE): each
            level processes all 3 k's in ONE instruction.
  P2: softmax: min/sum over the 9 neighbors via single tensor_reduce
      ops on a transposed [w,k] view of dist; sqrt/exp on Act.
  P3: products per (dy, c-half) fused over 3 dx (DVE; one on Pool in
      half 0), 10 accumulating matmuls per 16-channel group (PE) incl.
      the fe_lv residual, evacuation on Act, store DMA per half.
The two halves pipeline: engines overlap half-0 P3 with half-1 P1.
"""

import sys

if "/opt/trn_rl_repo" not in sys.path:
    sys.path.insert(0, "/opt/trn_rl_repo")

import os
from contextlib import ExitStack

import numpy as np

import concourse.bass as bass
import concourse.tile as tile
from concourse import mybir
from concourse.ap import AP as _AP
from concourse.masks import make_identity

B, C, H, W = 8, 64, 128, 128
N_CORES = 8
NH = 2                      # w-halves
WH = W // NH                # 64 cols per half
GW = WH + 2                 # stored cols incl 1 guard col each side
FP = mybir.dt.float32
BF = mybir.dt.bfloat16
ACT = mybir.ActivationFunctionType
X = mybir.AxisListType.X


def _split_sync_waits(nc, max_waits=1):
    """This container's walrus codegen accepts at most one sync-wait command
    per instruction; hoist extras into NoOps on the same engine."""
    for f in nc.m.functions:
        for blk in f.blocks:
            new_insts = []
            changed = False
            for inst in blk.instructions:
                si = getattr(inst, "sync_info", None)
                if si is not None and si.on_wait and len(si.on_wait) > max_waits:
                    waits = list(si.on_wait)
                    for i, w in enumerate(waits[max_waits:]):
                        nop = mybir.InstNoOp(
                            name=f"{inst.name}_ws{i}",
                            engine=inst.engine,
                            sync_info=mybir.SyncInfo(on_wait=[w],
                                                     on_update=[]),
                            bass_nofuse=True,
                        )
                        new_insts.append(nop)
                    inst.sync_info = mybir.SyncInfo(
                        on_wait=waits[:max_waits],
                        on_update=list(si.on_update),
                    )
                    changed = True
                new_insts.append(inst)
            if changed:
                blk.instructions = new_insts
    return nc


def _f3(ftile, c0, cg):
    """[128, 3(dx), cg, WH] view of a guarded [128, C, GW] tile: element
    (p, dx, c, w) = ftile[p, c0+c, w+dx+1]; dx and w share stride 1."""
    base = ftile[:, c0:c0 + cg, 1:1 + WH]
    bap = [list(x) for x in list(base.ap)]
    return _AP(base.tensor, base.offset - 1,
               [bap[0], [1, 3], bap[1], bap[2]])


def _tview(ap3, inner):
    """Swap the two free dims of a [128, A, B] AP -> [128, B, A] so that
    tensor_reduce(axis=X) reduces the original MIDDLE axis."""
    dims = [list(x) for x in list(ap3.ap)]
    assert len(dims) == 3
    return _AP(ap3.tensor, ap3.offset, [dims[0], dims[2], dims[1]])


def _build_kernel(split_waits=True):
    nc = bass.Bass("TRN2", target_bir_lowering=False, debug=False,
                   num_devices=N_CORES)

    x_d = [nc.dram_tensor(f"x{h}", [H, C * WH], BF, kind="ExternalInput").ap()
           for h in range(NH)]
    f_d = [nc.dram_tensor(f"f{h}", [H, C * GW], BF, kind="ExternalInput").ap()
           for h in range(NH)]
    fm_d = [nc.dram_tensor(f"fm{h}", [H, C * GW], BF,
                           kind="ExternalInput").ap() for h in range(NH)]
    fp_d = [nc.dram_tensor(f"fp{h}", [H, C * GW], BF,
                           kind="ExternalInput").ap() for h in range(NH)]
    o_d = [nc.dram_tensor(f"o{h}", [H, C * WH], BF,
                          kind="ExternalOutput").ap() for h in range(NH)]

    with tile.TileContext(nc) as tc, ExitStack() as ctx:
        main = ctx.enter_context(tc.tile_pool(name="main", bufs=1))
        tp = ctx.enter_context(tc.tile_pool(name="tp", bufs=3))
        tp2 = ctx.enter_context(tc.tile_pool(name="tp2", bufs=4))
        fin = ctx.enter_context(tc.tile_pool(name="fin", bufs=1))
        pst = ctx.enter_context(tc.tile_pool(name="pst", bufs=2,
                                             space="PSUM"))
        psa = ctx.enter_context(tc.tile_pool(name="psa", bufs=3,
                                             space="PSUM"))

        ident = main.tile([128, 128], BF)
        make_identity(nc, ident[:, :])

        xh = [main.tile([128, C, WH], BF, name=f"xh{h}") for h in range(NH)]
        fh = [main.tile([128, C, GW], BF, name=f"fh{h}") for h in range(NH)]
        fmh = [main.tile([128, C, GW], BF, name=f"fmh{h}") for h in range(NH)]
        fph = [main.tile([128, C, GW], BF, name=f"fph{h}") for h in range(NH)]
        outh = [main.tile([128, C, WH], BF, name=f"outh{h}")
                for h in range(NH)]
        dist = [main.tile([128, 9, WH], FP, name=f"dist{h}")
                for h in range(NH)]
        ewb = [main.tile([128, 9, WH], BF, name=f"ewb{h}") for h in range(NH)]
        mmin = [main.tile([128, WH], FP, name=f"mmin{h}") for h in range(NH)]
        ssum = [main.tile([128, WH], FP, name=f"ssum{h}") for h in range(NH)]

        # ---- loads: half-0 tiles first; 3 trigger queues ----
        # half-0 spread over all 3 trigger queues, c-halved so the first
        # (c-split) subtract can start as soon as the first halves land
        nc.sync.dma_start(out=xh[0][:, 0:32, :], in_=x_d[0][:, 0:32 * WH])
        nc.scalar.dma_start(out=fh[0][:, 0:32, :], in_=f_d[0][:, 0:32 * GW])
        nc.gpsimd.dma_start(out=fmh[0][:, :, :], in_=fm_d[0])
        nc.sync.dma_start(out=xh[0][:, 32:64, :], in_=x_d[0][:, 32 * WH:])
        nc.scalar.dma_start(out=fh[0][:, 32:64, :], in_=f_d[0][:, 32 * GW:])
        nc.gpsimd.dma_start(out=fph[0][:, :, :], in_=fp_d[0])
        nc.sync.dma_start(out=xh[1][:, :, :], in_=x_d[1])
        nc.sync.dma_start(out=fh[1][:, :, :], in_=f_d[1])
        nc.gpsimd.dma_start(out=fmh[1][:, :, :], in_=fm_d[1])
        nc.scalar.dma_start(out=fph[1][:, :, :], in_=fp_d[1])

        f_dy = lambda h: {-1: fmh[h], 0: fh[h], 1: fph[h]}

        def tree_pe(h, k, sqk):
            """sum over C: 8 accumulating 512-col matmuls + 1-op finish."""
            p = pst.tile([128, 8, WH], FP, tag="pst", name="ptree")
            flat = sqk.rearrange("p c w -> p (c w)")
            pf = p[:, :, :].rearrange("p c w -> p (c w)")
            for j in range(8):
                nc.tensor.matmul(
                    pf, ident[:, :],
                    flat[:, j * 8 * WH:(j + 1) * 8 * WH],
                    start=(j == 0), stop=(j == 7),
                )
            nc.vector.tensor_reduce(dist[h][:, k, :], _tview(p[:, :, :], 8),
                                    axis=X, op=mybir.AluOpType.add)

        def tree3_dve(h, k0, sq3):
            """fused 3-neighbor halving tree over c: each level handles all
            3 k's in one instruction; last level writes fp32 dist rows."""
            s = fin.tile([128, 3, 32, WH], BF, tag="s3", name="s3")
            nc.vector.tensor_add(s[:, :, :, :], sq3[:, :, 0:32, :],
                                 sq3[:, :, 32:64, :])
            n = 16
            while n >= 2:
                nc.vector.tensor_add(s[:, :, 0:n, :], s[:, :, 0:n, :],
                                     s[:, :, n:2 * n, :])
                n //= 2
            nc.vector.tensor_add(dist[h][:, k0:k0 + 3, :], s[:, :, 0, :],
                                 s[:, :, 1, :])

        def p1_group(h, dy, split_first=False):
            k0 = (dy + 1) * 3
            t = tp.tile([128, 3, C, WH], BF, tag="t", name="tsub")
            xb = xh[h][:, :, :].unsqueeze(1).broadcast_to([128, 3, C, WH])
            if split_first:
                # c-halved sub+square so compute starts before the full
                # f/x tiles have streamed in
                for cs in range(2):
                    ca, cb = cs * 32, cs * 32 + 32
                    xbh = (xh[h][:, ca:cb, :].unsqueeze(1)
                           .broadcast_to([128, 3, 32, WH]))
                    nc.vector.tensor_sub(t[:, :, ca:cb, :],
                                         _f3(f_dy(h)[dy], ca, 32), xbh)
                    nc.scalar.activation(t[:, :, ca:cb, :],
                                         t[:, :, ca:cb, :], ACT.Square)
            else:
                nc.vector.tensor_sub(t[:, :, :, :], _f3(f_dy(h)[dy], 0, C),
                                     xb)
                # Act squares 56 channels; DVE self-multiplies the last 8
                # (fills DVE's per-group slack, shortens the Act period)
                nc.scalar.activation(t[:, :, 0:56, :], t[:, :, 0:56, :],
                                     ACT.Square)
                nc.vector.tensor_mul(t[:, :, 56:64, :], t[:, :, 56:64, :],
                                     t[:, :, 56:64, :])
            for kk in range(3):
                tree_pe(h, k0 + kk, t[:, kk, :, :])

        def p2(h):
            d, mn, sm = dist[h], mmin[h], ssum[h]
            nc.vector.tensor_reduce(mn[:, :], _tview(d[:, :, :], 9),
                                    axis=X, op=mybir.AluOpType.min)
            nc.scalar.activation(d[:, :, :], d[:, :, :], ACT.Sqrt)
            nc.scalar.activation(mn[:, :], mn[:, :], ACT.Sqrt)
            nc.vector.tensor_sub(
                d[:, :, :],
                mn[:, :].unsqueeze(1).broadcast_to([128, 9, WH]),
                d[:, :, :],
            )
            nc.scalar.activation(d[:, :, :], d[:, :, :], ACT.Exp)
            nc.vector.tensor_reduce(sm[:, :], _tview(d[:, :, :], 9),
                                    axis=X, op=mybir.AluOpType.add)
            nc.vector.reciprocal(sm[:, :], sm[:, :])
            nc.vector.tensor_mul(
                ewb[h][:, :, :], d[:, :, :],
                sm[:, :].unsqueeze(1).broadcast_to([128, 9, WH]),
            )

        def p3(h):
            fd = f_dy(h)
            for g in range(4):               # 16-channel groups
                c0 = g * 16
                t2s = []
                for dyi, dy in enumerate((-1, 0, 1)):
                    k0 = dyi * 3
                    e3 = (ewb[h][:, k0:k0 + 3, :]
                          .unsqueeze(2).broadcast_to([128, 3, 16, WH]))
                    t2 = tp2.tile([128, 3, 16, WH], BF, tag="t2", name="t2")
                    nc.vector.tensor_mul(t2[:, :, :, :],
                                         _f3(fd[dy], c0, 16), e3)
                    t2s.append(t2)
                pacc = psa.tile([128, 16 * WH], FP, tag="pacc", name="pacc")
                # (dyi, kk) outermost so each t2 is fully consumed (and its
                # pool slot released) after its 6 matmuls
                for dyi in range(3):
                    for kk in range(3):
                        for cc in range(2):  # 512-col chunks (8c each)
                            nc.tensor.matmul(
                                pacc[:, cc * 512:(cc + 1) * 512], ident[:, :],
                                (t2s[dyi][:, kk, cc * 8:cc * 8 + 8, :]
                                 .rearrange("p c w -> p (c w)")),
                                start=(dyi == 0 and kk == 0), stop=False,
                            )
                for cc in range(2):
                    nc.tensor.matmul(
                        pacc[:, cc * 512:(cc + 1) * 512], ident[:, :],
                        (xh[h][:, c0 + cc * 8:c0 + cc * 8 + 8, :]
                         .rearrange("p c w -> p (c w)")),
                        start=False, stop=True,
                    )
                nc.scalar.activation(
                    (outh[h][:, c0:c0 + 16, :]
                     .rearrange("p c w -> p (c w)")),
                    pacc[:, :], ACT.Copy)
                if g % 2 == 1:   # store as soon as each 32-channel pair done
                    nc.sync.dma_start(
                        out=o_d[h][:, (g - 1) * 16 * WH:(g + 1) * 16 * WH],
                        in_=(outh[h][:, (g - 1) * 16:(g + 1) * 16, :]
                             .rearrange("p c w -> p (c w)")))

        for h in range(NH):
            order = (0, -1, 1) if h == 0 else (-1, 0, 1)
            for i, dy in enumerate(order):
                p1_group(h, dy, split_first=(h == 0 and i == 0))
            p2(h)
            p3(h)

    return _split_sync_waits(nc) if split_waits else nc


_cache = {}


class _SpmdRunner:
    """Executes the Bass graph SPMD on the 8 cores via PJRT/shard_map."""

    def __init__(self, nc, n_cores):
        import jax
        from jax.experimental.shard_map import shard_map
        from jax.sharding import Mesh, NamedSharding, PartitionSpec

        from concourse import bass2jax as b2j

        b2j.install_neuronx_cc_hook()
        self.nc = nc
        self.n_cores = n_cores
        partition_name = (
            nc.partition_id_tensor.name if nc.partition_id_tensor else None
        )

        in_names, out_names, out_avals = [], [], []
        for alloc in nc.m.functions[0].allocations:
            if not isinstance(alloc, mybir.MemoryLocationSet):
                continue
            name = alloc.memorylocations[0].name
            if alloc.kind == "ExternalInput":
                if name != partition_name:
                    in_names.append(name)
            elif alloc.kind == "ExternalOutput":
                out_names.append(name)
                out_avals.append(
                    jax.core.ShapedArray(
                        tuple(alloc.tensor_shape), mybir.dt.np(alloc.dtype)
                    )
                )
        self.in_names, self.out_names = in_names, out_names
        self.out_avals = out_avals
        n_params, n_outs = len(in_names), len(out_names)
        all_in_names = in_names + out_names + (
            [partition_name] if partition_name else []
        )

        def _body(*args):
            operands = list(args)
            if partition_name is not None:
                operands.append(b2j.partition_id_tensor())
            outs = b2j._bass_exec_p.bind(
                *operands,
                out_avals=tuple(out_avals),
                in_names=tuple(all_in_names),
                out_names=tuple(out_names),
                lowering_input_output_aliases=(),
                sim_require_finite=True,
                sim_require_nnan=True,
                nc=nc,
            )
            return tuple(outs)

        self.devices = jax.devices()[:n_cores]
        assert len(self.devices) == n_cores
        mesh = Mesh(np.asarray(self.devices), ("core",))
        self.sharding = NamedSharding(mesh, PartitionSpec("core"))
        self.sharded = jax.jit(
            shard_map(
                _body, mesh=mesh,
                in_specs=(PartitionSpec("core"),) * (n_params + n_outs),
                out_specs=(PartitionSpec("core"),) * n_outs,
                check_rep=False,
            ),
            donate_argnums=tuple(range(n_params, n_params + n_outs)),
            keep_unused=True,
        )

    def _make_global(self, shards_np):
        import jax

        shards = [
            jax.device_put(s, self.devices[c])
            for c, s in enumerate(shards_np)
        ]
        gshape = (self.n_cores * shards_np[0].shape[0],) + tuple(
            shards_np[0].shape[1:]
        )
        return jax.make_array_from_single_device_arrays(
            gshape, self.sharding, shards
        )

    def __call__(self, in_maps):
        gin = [
            self._make_global(
                [np.asarray(in_maps[c][name]) for c in range(self.n_cores)]
            )
            for name in self.in_names
        ]
        gzero = [
            self._make_global(
                [np.zeros(a.shape, a.dtype) for _ in range(self.n_cores)]
            )
            for a in self.out_avals
        ]
        out_arrs = self.sharded(*gin, *gzero)
        results = [dict() for _ in range(self.n_cores)]
        for i, name in enumerate(self.out_names):
            for sh in out_arrs[i].addressable_shards:
                results[self.devices.index(sh.device)][name] = np.asarray(
                    sh.data
                )
        return results




def _get_runner():
    if "runner" not in _cache:
        _cache["runner"] = _SpmdRunner(_build_kernel(), N_CORES)
    return _cache["runner"]


def _prep_inputs(fe_lv, fused_features):
    import ml_dtypes

    bf = ml_dtypes.bfloat16
    fe_lv = np.asarray(fe_lv, dtype=np.float32)
    fused_features = np.asarray(fused_features, dtype=np.float32)
    in_maps = []
    for i in range(N_CORES):
        x = np.ascontiguousarray(fe_lv[i].transpose(1, 0, 2)).astype(bf)
        f = np.ascontiguousarray(
            fused_features[i].transpose(1, 0, 2)).astype(bf)   # [H, C, W]
        # h-shifted variants with ZERO rows at the global h edges
        fm = np.concatenate([np.zeros_like(f[0:1]), f[:-1]], axis=0)
        fp = np.concatenate([f[1:], np.zeros_like(f[-1:])], axis=0)
        # guarded [H, C, W+2] with zero cols at the global w edges
        def _guard(a):
            g = np.zeros((H, C, W + 2), dtype=bf)
            g[:, :, 1:W + 1] = a
            return g
        fg, fmg, fpg = _guard(f), _guard(fm), _guard(fp)
        m = {}
        for h in range(NH):
            w0 = h * WH
            m[f"x{h}"] = np.ascontiguousarray(
                x[:, :, w0:w0 + WH]).reshape(H, C * WH)
            m[f"f{h}"] = np.ascontiguousarray(
                fg[:, :, w0:w0 + GW]).reshape(H, C * GW)
            m[f"fm{h}"] = np.ascontiguousarray(
                fmg[:, :, w0:w0 + GW]).reshape(H, C * GW)
            m[f"fp{h}"] = np.ascontiguousarray(
                fpg[:, :, w0:w0 + GW]).reshape(H, C * GW)
        in_maps.append(m)
    return in_maps


def _post_outputs(results):
    outs = []
    for i in range(N_CORES):
        hs = [results[i][f"o{h}"].reshape(H, C, WH) for h in range(NH)]
        hcw = np.concatenate(hs, axis=2)           # [H, C, W]
        outs.append(hcw.transpose(1, 0, 2))        # [C, H, W]
    return np.ascontiguousarray(np.stack(outs, axis=0)).astype(np.float32)


def kernel(fe_lv, fused_features):
    runner = _get_runner()
    results = runner(_prep_inputs(fe_lv, fused_features))
    return _post_outputs(results)

def bench(fe_lv, fused_features, trace_dir=None):
    """Run once (compiling/warming), then re-run under an NTFF profile
    capture and return (out, exec_time_ns, trace_info)."""
    import ctypes
    import glob as _glob
    import tempfile

    out = kernel(fe_lv, fused_features)
    runner = _cache["runner"]

    neff_dir = trace_dir or tempfile.mkdtemp(prefix="ntff_prof_")
    lib = ctypes.CDLL("/opt/axon/libaxon_pjrt.so")
    if not hasattr(lib, "axon_start_nrt_profile"):
        return out, None, "no axon_start_nrt_profile symbol"
    lib.axon_start_nrt_profile.argtypes = [
        ctypes.POINTER(ctypes.c_int64), ctypes.c_size_t,
    ]
    lib.axon_start_nrt_profile.restype = ctypes.c_int64
    lib.axon_stop_nrt_profile.argtypes = [ctypes.c_char_p]
    lib.axon_stop_nrt_profile.restype = ctypes.c_int64

    in_maps = _prep_inputs(fe_lv, fused_features)
    rc = lib.axon_start_nrt_profile(None, 0)
    if rc != 0:
        return out, None, f"axon_start_nrt_profile rc={rc}"
    runner(in_maps)
    n = lib.axon_stop_nrt_profile(neff_dir.encode())
    if n <= 0:
        return out, None, f"axon_stop_nrt_profile rc={n}"

    ntffs = _glob.glob(os.path.join(neff_dir, "*_body*.ntff"))
    if not ntffs:
        return out, None, f"no *_body*.ntff in {neff_dir}: " + str(
            sorted(os.listdir(neff_dir)))

    import gauge.profiler
    from concourse._compat import FishPath

    profile = gauge.profiler.Profile(
        profile_path=FishPath(neff_dir),
        kernel_dev_mode=True,
        profile_on_exit=False,
        bass_kernel=_cache["runner"].nc.m,
        offline_processing=True,
        fname="*_body*",
    )
    perfetto_results = profile.to_perfetto(model_index=(0,))
    if not perfetto_results:
        return out, None, f"no perfetto results ({neff_dir})"
    pr = perfetto_results[0]
    return out, pr.exec_time_ns, {"trace_path": pr.trace_path,
                                  "neff_dir": neff_dir}
